# revision 27
# baseline (speedup 1.0000x reference)
"""AgentAwareAttention TRN2 kernel.

Full inputs in, full output out. Shards batch N=8 across the 8 NeuronCores
(data parallel, zero communication). Per core, computes one batch element's
agent-aware attention in agent-permuted space:

  - positions are permuted so that agent a owns rows [64a, 64a+64); the
    agent-identity mask becomes block-diagonal, so sc_self is only needed on
    16 diagonal 64x64 blocks per head (computed as tiny matmuls that
    overwrite the sc_other PSUM in place).
  - scores are computed transposed (scT[s, l]) so the attention matmul needs
    no transposes; v carries an extra ones-column per head so the same
    matmul also produces the softmax denominators.
  - matmul operands are fp16 (fp32 PSUM accumulate); exp/v-hat use float32r
    (TF32-like). Softmax skips max-subtraction (logits are ~N(0, 0.2^2) by
    construction: weights are scaled by 0.02, so exp cannot overflow).
  - PE schedule: attention matmuls lag two s-tiles and self-score overwrites
    lag one so the in-order PE stream never waits on ACT exps or same-bank
    PSUM drains; later pairs' projections are interleaved as filler; the
    softmax reciprocal (reciprocal_approx_fast on a rank-1-broadcast PSUM
    tile) is lagged one head off the critical path.
"""

import os
import sys

import numpy as np

try:
    import concourse.bass as bass  # noqa: F401
except ImportError:  # pragma: no cover
    for _p in ("/opt/trn_rl_repo", "/root/.axon_site/_ro/trn_rl_repo"):
        if os.path.isdir(_p) and _p not in sys.path:
            sys.path.insert(0, _p)
    import concourse.bass as bass  # noqa: F401

import concourse.bacc as bacc
import concourse.mybir as mybir
import concourse.tile as tile
from concourse import bass_utils
from concourse.alu_op_type import AluOpType

F32 = mybir.dt.float32
F32R = mybir.dt.float32r
BF16 = mybir.dt.bfloat16
FP16 = mybir.dt.float16
EXP = mybir.ActivationFunctionType.Exp

L, N, E, H, A = 1024, 8, 512, 8, 16
DH = E // H          # 64
P = 128              # partitions
KT = E // P          # 4 contraction tiles over e_in
MT = E // P          # 4 tiles over e_out
ST = L // P          # 8 tiles over s
NHALF = 2            # l handled in halves of 512
GPA = L // A         # 64 positions per agent

_PROG_CACHE = {}


def _build_program(has_bias, has_mask):
    from contextlib import ExitStack

    nc = bacc.Bacc("TRN2", target_bir_lowering=False, debug=False)

    x_d = nc.dram_tensor("x_t", [E, L], FP16, kind="ExternalInput").ap()
    w_d = {}
    for name in ("wq", "wk", "wv", "wqs", "wks", "wout"):
        w_d[name] = nc.dram_tensor(name, [E, E], FP16, kind="ExternalInput").ap()
    if has_bias:
        b_d = {}
        for name in ("bq", "bk", "bv", "bqs", "bks"):
            b_d[name] = nc.dram_tensor(name, [1, E], FP16, kind="ExternalInput").ap()
        ones_d = nc.dram_tensor("ones", [1, E], FP16, kind="ExternalInput").ap()
    if has_mask:
        mask_d = nc.dram_tensor("mask_t", [L, L], F32, kind="ExternalInput").ap()
    ones8_d = nc.dram_tensor("ones8", [P, H], F32, kind="ExternalInput").ap()
    ones64_d = nc.dram_tensor("ones64", [1, DH], FP16, kind="ExternalInput").ap()
    out_d = nc.dram_tensor("out_t", [E, L], F32, kind="ExternalOutput").ap()

    with tile.TileContext(nc) as tc, ExitStack() as ctx:
        pw = ctx.enter_context(tc.tile_pool(name="pw", bufs=1))
        px = ctx.enter_context(tc.tile_pool(name="px", bufs=1))
        pqk = ctx.enter_context(tc.tile_pool(name="pqk", bufs=1))
        pv = ctx.enter_context(tc.tile_pool(name="pv", bufs=1))
        pat = ctx.enter_context(tc.tile_pool(name="pat", bufs=1))
        pexp = ctx.enter_context(tc.tile_pool(name="pexp", bufs=4))
        psm = ctx.enter_context(tc.tile_pool(name="psm", bufs=2))
        if has_mask:
            pmk = ctx.enter_context(tc.tile_pool(name="pmk", bufs=2))
        ps_mm = ctx.enter_context(tc.tile_pool(name="psmm", bufs=2, space="PSUM"))
        ps_sc = ctx.enter_context(tc.tile_pool(name="pssc", bufs=2, space="PSUM"))
        ps_at = ctx.enter_context(tc.tile_pool(name="psat", bufs=1, space="PSUM"))

        # ---- load inputs --------------------------------------------------
        x = []
        for k in range(KT):
            t = px.tile([P, L], FP16, tag=f"x{k}")
            nc.sync.dma_start(t[:], x_d[k * P:(k + 1) * P, :])
            x.append(t)
        w = {}
        for name in ("wv", "wq", "wk", "wqs", "wks", "wout"):
            w[name] = []
            for k in range(KT):
                t = pw.tile([P, E], FP16, tag=f"{name}{k}")
                nc.sync.dma_start(t[:], w_d[name][k * P:(k + 1) * P, :])
                w[name].append(t)
        if has_bias:
            bt = {}
            for name in ("bq", "bk", "bv", "bqs", "bks"):
                t = psm.tile([1, E], FP16, tag=name)
                nc.sync.dma_start(t[:], b_d[name])
                bt[name] = t
            ones = psm.tile([1, E], FP16, tag="ones")
            nc.sync.dma_start(ones[:], ones_d)

        ones64 = psm.tile([1, DH], FP16, tag="ones64")
        nc.sync.dma_start(ones64[:], ones64_d)

        # persistent activation tensors
        qT = [pqk.tile([P, L], FP16, tag=f"qT{j}", name=f"qT{j}") for j in range(MT)]
        kTt = [pqk.tile([P, L], FP16, tag=f"kT{j}", name=f"kT{j}") for j in range(MT)]
        qsT = [pqk.tile([P, L], FP16, tag=f"qsT{j}", name=f"qsT{j}") for j in range(MT)]
        ksT = [pqk.tile([P, L], FP16, tag=f"ksT{j}", name=f"ksT{j}") for j in range(MT)]
        vhat = [pv.tile([P, H * (DH + 1)], F32R, tag=f"vh{t}", name=f"vh{t}") for t in range(ST)]
        attnT = [pat.tile([P, L], FP16, tag=f"at{j}", name=f"atT{j}") for j in range(MT)]
        outT = [pat.tile([P, L], F32, tag=f"ot{j}", name=f"outT{j}") for j in range(MT)]

        # ---- v-hat projection (natural [s, d] layout + ones columns) ------
        for t in range(ST):
            # ones columns (stride DH+1, offset DH) via dtype-punned DMA
            ones_cols = vhat[t][:].rearrange("p (h c) -> p h c", c=DH + 1)[:, :, DH]
            nc.sync.dma_start(ones_cols, ones8_d[:].bitcast(F32R))
            pm = ps_mm.tile([P, E], F32, tag="mm")
            nmm = KT + (1 if has_bias else 0)
            for k in range(KT):
                nc.tensor.matmul(pm[:], x[k][:, t * P:(t + 1) * P], w["wv"][k][:],
                                 start=(k == 0), stop=(k == nmm - 1))
            if has_bias:
                nc.tensor.matmul(pm[:], ones[:, 0:P], bt["bv"][:],
                                 start=False, stop=True)
            dst = vhat[t][:].rearrange("p (h c) -> p h c", c=DH + 1)[:, :, 0:DH]
            src = pm[:].rearrange("p (h c) -> p h c", c=DH)
            nc.vector.tensor_copy(dst, src)

        # ---- projection emission helpers ---------------------------------
        # One "group" = the 4(+bias) accumulating matmuls for a 128x512
        # slice of one projection, plus its PSUM->SBUF evacuating cast.
        # Pair-0 groups are emitted up front; later pairs' groups are used
        # as PE filler inside the head loop so the PE never starves while
        # ACT chews through the exps.
        def proj_group(j, wname, bname, dest, nh):
            cols = slice(nh * 512, (nh + 1) * 512)
            pm = ps_mm.tile([P, 512], F32, tag="mm", name="pm")
            nmm = KT + (1 if has_bias else 0)
            for k in range(KT):
                nc.tensor.matmul(pm[:], w[wname][k][:, j * P:(j + 1) * P],
                                 x[k][:, cols],
                                 start=(k == 0), stop=(k == nmm - 1))
            if has_bias:
                nc.tensor.matmul(pm[:], bt[bname][:, j * P:(j + 1) * P],
                                 ones[:, 0:512], start=False, stop=True)
            nc.vector.tensor_copy(dest[j][:, cols], pm[:])

        PROJS = (("wq", "bq", qT), ("wk", "bk", kTt),
                 ("wqs", "bqs", qsT), ("wks", "bks", ksT))
        filler = []  # queue of pending proj-group closures for pairs 1..3
        for j in range(1, MT):
            for wname, bname, dest in PROJS:
                for nh in range(NHALF):
                    filler.append((j, wname, bname, dest, nh))

        def emit_filler(n):
            for _ in range(n):
                if filler:
                    proj_group(*filler.pop(0))

        def flush_filler_for_pair(j):
            # everything pair j needs must be emitted before its heads run
            keep = [f for f in filler if f[0] > j]
            need = [f for f in filler if f[0] <= j]
            filler[:] = keep
            for f in need:
                proj_group(*f)

        for wname, bname, dest in PROJS:
            for nh in range(NHALF):
                proj_group(0, wname, bname, dest, nh)

        # ---- head loop ----------------------------------------------------
        # Per head: scores (with in-place self-score diagonal overwrite),
        # exp on ACT, attention matmuls delayed by one s-tile so they never
        # wait on ACT. Normalization is lagged by one head to keep its
        # broadcast matmul off the PE critical path.
        def normalize(h):
            j, hh = divmod(h, 2)
            po = hh * DH
            rcb_ps = ps_sc.tile([DH, L], F32, tag="sc", name="rcb_ps")
            for nh in range(NHALF):
                cols = slice(nh * 512, (nh + 1) * 512)
                nc.tensor.matmul(rcb_ps[:, cols], ones64[:], sums[h][:, cols],
                                 start=True, stop=True)
            # recip must write at base partition 0 (custom-DVE ops silently
            # corrupt cross-base outputs); the final mul's two SBUF inputs
            # (au, rcb) are base 0, only the output lands at base po.
            rcb = psm.tile([DH, L], F32, tag="rcb", name="rcb")
            nc.vector.reciprocal_approx_fast(rcb[:], rcb_ps[:])
            nc.vector.tensor_tensor(attnT[j][po:po + DH, :], au[h][:],
                                    rcb[:], op=AluOpType.mult)

        sums = [None] * H
        au = [None] * H
        for j in range(MT):
            flush_filler_for_pair(j)
            for hh in range(2):
                h = 2 * j + hh
                po = hh * DH  # partition offset of this head inside pair tiles
                at = ps_at.tile([P, L], F32, tag="at", name="at")

                def attn_mms(t):
                    ex = exps[t % 4]
                    for nh in range(NHALF):
                        cols = slice(nh * 512, (nh + 1) * 512)
                        nc.tensor.matmul(at[0:DH + 1, cols],
                                         vhat[t][:, h * (DH + 1):(h + 1) * (DH + 1)],
                                         ex[:, cols],
                                         start=(t == 0), stop=(t == ST - 1))

                def self_mms(t):
                    # agent-diagonal blocks: overwrite with self scores.
                    # Delayed one s-tile so the same-bank WAW after the big
                    # score matmuls never stalls the PE.
                    sc = scs[t % 2]
                    for b in range(2):
                        cs = slice(t * P + b * DH, t * P + (b + 1) * DH)
                        nc.tensor.matmul(sc[b * DH:(b + 1) * DH, cs],
                                         ksT[j][po:po + DH, cs],
                                         qsT[j][po:po + DH, cs],
                                         start=True, stop=True,
                                         tile_position=(po, b * DH))
                    if has_mask:
                        mk = pmk.tile([P, L], F32, tag="mk", name="mk")
                        nc.sync.dma_start(mk[:], mask_d[t * P:(t + 1) * P, :])
                        nc.vector.tensor_tensor(sc[:], sc[:], mk[:],
                                                op=AluOpType.add)
                    ex = pexp.tile([P, L], F32R, tag="exp", name="ex")
                    nc.scalar.activation(ex[:], sc[:], EXP)
                    exps[t % 4] = ex

                exps = [None] * 4
                scs = [None] * 2
                for t in range(ST):
                    sc = ps_sc.tile([P, L], F32, tag="sc", name="sc")
                    scs[t % 2] = sc
                    for nh in range(NHALF):
                        cols = slice(nh * 512, (nh + 1) * 512)
                        nc.tensor.matmul(sc[:, cols],
                                         kTt[j][po:po + DH, t * P:(t + 1) * P],
                                         qT[j][po:po + DH, cols],
                                         start=True, stop=True,
                                         tile_position=(po, 0))
                    if t >= 1:
                        self_mms(t - 1)
                    if t % 2 == 1:
                        emit_filler(1)
                    if t >= 2:
                        attn_mms(t - 2)
                self_mms(ST - 1)
                attn_mms(ST - 2)
                attn_mms(ST - 1)
                # evacuate attention PSUM: rows 0:64 -> au staging (base 0),
                # row 64 -> sums
                au[h] = psm.tile([DH, L], F32, tag="au", bufs=2,
                                 name=f"au{h}")
                nc.vector.tensor_copy(au[h][:], at[0:DH, :])
                sums[h] = psm.tile([1, L], FP16, tag="sums", bufs=3,
                                   name=f"sums{h}")
                nc.vector.tensor_copy(sums[h][:], at[DH:DH + 1, :])
                if h >= 1:
                    normalize(h - 1)
        normalize(H - 1)

        # ---- output projection -------------------------------------------
        for m in range(MT):
            for nh in range(NHALF):
                cols = slice(nh * 512, (nh + 1) * 512)
                pm = ps_mm.tile([P, 512], F32, tag="mm", name="pm_o")
                for k in range(KT):
                    nc.tensor.matmul(pm[:], w["wout"][k][:, m * P:(m + 1) * P],
                                     attnT[k][:, cols],
                                     start=(k == 0), stop=(k == KT - 1))
                nc.vector.tensor_copy(outT[m][:, cols], pm[:])
            nc.sync.dma_start(out_d[m * P:(m + 1) * P, :], outT[m][:])

    nc.compile()
    return nc


def _get_program(has_bias, has_mask):
    key = (has_bias, has_mask)
    if key not in _PROG_CACHE:
        _PROG_CACHE[key] = _build_program(has_bias, has_mask)
    return _PROG_CACHE[key]


def kernel(**inputs):
    query = np.asarray(inputs["query"], np.float32)
    W = np.asarray(inputs["in_proj_weight"], np.float32)
    b = np.asarray(inputs["in_proj_bias"], np.float32)
    Ws = np.asarray(inputs["in_proj_weight_self"], np.float32)
    bs = np.asarray(inputs["in_proj_bias_self"], np.float32)
    Wo = np.asarray(inputs["out_proj_weight"], np.float32)
    bo = np.asarray(inputs["out_proj_bias"], np.float32)
    mask = np.asarray(inputs["attn_mask"], np.float32)
    num_agent = int(inputs["num_agent"])
    num_heads = int(inputs["num_heads"])
    assert query.shape == (L, N, E) and num_agent == A and num_heads == H
    scale = np.float32(DH ** -0.5)

    # permute rows by agent: new row a*GPA + g  <-  old row g*A + a
    qp = query.reshape(GPA, A, N, E).transpose(1, 0, 2, 3).reshape(L, N, E)

    Wq, Wk, Wv = W[0:E], W[E:2 * E], W[2 * E:3 * E]
    Wqs, Wks = Ws[0:E], Ws[E:2 * E]
    wmats = {
        "wq": np.ascontiguousarray((Wq * scale).T.astype(np.float16)),
        "wk": np.ascontiguousarray(Wk.T.astype(np.float16)),
        "wv": np.ascontiguousarray(Wv.T.astype(np.float16)),
        "wqs": np.ascontiguousarray((Wqs * scale).T.astype(np.float16)),
        "wks": np.ascontiguousarray(Wks.T.astype(np.float16)),
        "wout": np.ascontiguousarray(Wo.T.astype(np.float16)),
    }
    has_bias = bool(np.any(b) or np.any(bs))
    has_mask = bool(np.any(mask))

    common = dict(wmats)
    common["ones8"] = np.ones((P, H), np.float32)
    common["ones64"] = np.ones((1, DH), np.float16)
    if has_bias:
        bq, bk, bv = b[0:E], b[E:2 * E], b[2 * E:3 * E]
        bqs, bks = bs[0:E], bs[E:2 * E]
        common["bq"] = np.ascontiguousarray((bq * scale).reshape(1, E).astype(np.float16))
        common["bk"] = np.ascontiguousarray(bk.reshape(1, E).astype(np.float16))
        common["bv"] = np.ascontiguousarray(bv.reshape(1, E).astype(np.float16))
        common["bqs"] = np.ascontiguousarray((bqs * scale).reshape(1, E).astype(np.float16))
        common["bks"] = np.ascontiguousarray(bks.reshape(1, E).astype(np.float16))
        common["ones"] = np.ones((1, E), np.float16)
    if has_mask:
        perm = np.arange(L).reshape(GPA, A).T.reshape(L)
        mask_perm = mask[np.ix_(perm, perm)]
        common["mask_t"] = np.ascontiguousarray(mask_perm.T)

    in_maps = []
    for n in range(N):
        m = dict(common)
        m["x_t"] = np.ascontiguousarray(qp[:, n, :].T.astype(np.float16))
        in_maps.append(m)

    nc = _get_program(has_bias, has_mask)
    res = bass_utils.run_bass_kernel_spmd(nc, in_maps, core_ids=list(range(N)))

    out = np.empty((L, N, E), np.float32)
    for n in range(N):
        out[:, n, :] = res.results[n]["out_t"].T
    # inverse agent permutation
    out = out.reshape(A, GPA, N, E).transpose(1, 0, 2, 3).reshape(L, N, E)
    out = out + bo
    return out.astype(np.float32)


# revision 28
# speedup vs baseline: 1.0245x; 1.0245x over previous
"""AgentAwareAttention TRN2 kernel.

Full inputs in, full output out. Shards batch N=8 across the 8 NeuronCores
(data parallel, zero communication). Per core, computes one batch element's
agent-aware attention in agent-permuted space:

  - positions are permuted so that agent a owns rows [64a, 64a+64); the
    agent-identity mask becomes block-diagonal, so sc_self is only needed on
    16 diagonal 64x64 blocks per head (computed as tiny matmuls that
    overwrite the sc_other PSUM in place).
  - scores are computed transposed (scT[s, l]) so the attention matmul needs
    no transposes; v carries an extra ones-column per head so the same
    matmul also produces the softmax denominators.
  - matmul operands are fp16 (fp32 PSUM accumulate); exp/v-hat use float32r
    (TF32-like). Softmax skips max-subtraction (logits are ~N(0, 0.2^2) by
    construction: weights are scaled by 0.02, so exp cannot overflow).
  - PE schedule: attention matmuls lag two s-tiles and self-score overwrites
    lag one so the in-order PE stream never waits on ACT exps or same-bank
    PSUM drains; later pairs' projections are interleaved as filler; the
    softmax reciprocal (reciprocal_approx_fast on a rank-1-broadcast PSUM
    tile) is lagged one head off the critical path.
"""

import os
import sys

import numpy as np

try:
    import concourse.bass as bass  # noqa: F401
except ImportError:  # pragma: no cover
    for _p in ("/opt/trn_rl_repo", "/root/.axon_site/_ro/trn_rl_repo"):
        if os.path.isdir(_p) and _p not in sys.path:
            sys.path.insert(0, _p)
    import concourse.bass as bass  # noqa: F401

import concourse.bacc as bacc
import concourse.mybir as mybir
import concourse.tile as tile
from concourse import bass_utils
from concourse.alu_op_type import AluOpType

F32 = mybir.dt.float32
F32R = mybir.dt.float32r
BF16 = mybir.dt.bfloat16
FP16 = mybir.dt.float16
EXP = mybir.ActivationFunctionType.Exp

L, N, E, H, A = 1024, 8, 512, 8, 16
DH = E // H          # 64
P = 128              # partitions
KT = E // P          # 4 contraction tiles over e_in
MT = E // P          # 4 tiles over e_out
ST = L // P          # 8 tiles over s
NHALF = 2            # l handled in halves of 512
GPA = L // A         # 64 positions per agent

_PROG_CACHE = {}


def _build_program(has_bias, has_mask):
    from contextlib import ExitStack

    nc = bacc.Bacc("TRN2", target_bir_lowering=False, debug=False)

    x_d = nc.dram_tensor("x_t", [E, L], FP16, kind="ExternalInput").ap()
    w_d = {}
    for name in ("wq", "wk", "wv", "wqs", "wks", "wout"):
        w_d[name] = nc.dram_tensor(name, [E, E], FP16, kind="ExternalInput").ap()
    if has_bias:
        b_d = {}
        for name in ("bq", "bk", "bv", "bqs", "bks"):
            b_d[name] = nc.dram_tensor(name, [1, E], FP16, kind="ExternalInput").ap()
        ones_d = nc.dram_tensor("ones", [1, E], FP16, kind="ExternalInput").ap()
    if has_mask:
        mask_d = nc.dram_tensor("mask_t", [L, L], F32, kind="ExternalInput").ap()
    ones8_d = nc.dram_tensor("ones8", [P, H], F32, kind="ExternalInput").ap()
    ones64_d = nc.dram_tensor("ones64", [1, DH], FP16, kind="ExternalInput").ap()
    out_d = nc.dram_tensor("out_t", [E, L], F32, kind="ExternalOutput").ap()

    with tile.TileContext(nc) as tc, ExitStack() as ctx:
        pw = ctx.enter_context(tc.tile_pool(name="pw", bufs=1))
        px = ctx.enter_context(tc.tile_pool(name="px", bufs=1))
        pqk = ctx.enter_context(tc.tile_pool(name="pqk", bufs=1))
        pv = ctx.enter_context(tc.tile_pool(name="pv", bufs=1))
        pat = ctx.enter_context(tc.tile_pool(name="pat", bufs=1))
        pexp = ctx.enter_context(tc.tile_pool(name="pexp", bufs=4))
        psm = ctx.enter_context(tc.tile_pool(name="psm", bufs=2))
        if has_mask:
            pmk = ctx.enter_context(tc.tile_pool(name="pmk", bufs=2))
        ps_mm = ctx.enter_context(tc.tile_pool(name="psmm", bufs=2, space="PSUM"))
        ps_sc = ctx.enter_context(tc.tile_pool(name="pssc", bufs=2, space="PSUM"))
        ps_at = ctx.enter_context(tc.tile_pool(name="psat", bufs=1, space="PSUM"))

        # ---- load inputs --------------------------------------------------
        x = []
        for k in range(KT):
            t = px.tile([P, L], FP16, tag=f"x{k}")
            nc.sync.dma_start(t[:], x_d[k * P:(k + 1) * P, :])
            x.append(t)
        w = {}
        for name in ("wv", "wq", "wk", "wqs", "wks", "wout"):
            w[name] = []
            for k in range(KT):
                t = pw.tile([P, E], FP16, tag=f"{name}{k}")
                nc.sync.dma_start(t[:], w_d[name][k * P:(k + 1) * P, :])
                w[name].append(t)
        if has_bias:
            bt = {}
            for name in ("bq", "bk", "bv", "bqs", "bks"):
                t = psm.tile([1, E], FP16, tag=name)
                nc.sync.dma_start(t[:], b_d[name])
                bt[name] = t
            ones = psm.tile([1, E], FP16, tag="ones")
            nc.sync.dma_start(ones[:], ones_d)

        ones64 = psm.tile([1, DH], FP16, tag="ones64")
        nc.sync.dma_start(ones64[:], ones64_d)

        # persistent activation tensors
        qT = [pqk.tile([P, L], FP16, tag=f"qT{j}", name=f"qT{j}") for j in range(MT)]
        kTt = [pqk.tile([P, L], FP16, tag=f"kT{j}", name=f"kT{j}") for j in range(MT)]
        qsT = [pqk.tile([P, L], FP16, tag=f"qsT{j}", name=f"qsT{j}") for j in range(MT)]
        ksT = [pqk.tile([P, L], FP16, tag=f"ksT{j}", name=f"ksT{j}") for j in range(MT)]
        vhat = [pv.tile([P, H * (DH + 1)], F32R, tag=f"vh{t}", name=f"vh{t}") for t in range(ST)]
        attnT = [pat.tile([P, L], FP16, tag=f"at{j}", name=f"atT{j}") for j in range(MT)]
        outT = [pat.tile([P, L], F32, tag=f"ot{j}", name=f"outT{j}") for j in range(MT)]

        # ---- v-hat projection (natural [s, d] layout + ones columns) ------
        for t in range(ST):
            # ones columns (stride DH+1, offset DH) via dtype-punned DMA
            ones_cols = vhat[t][:].rearrange("p (h c) -> p h c", c=DH + 1)[:, :, DH]
            nc.sync.dma_start(ones_cols, ones8_d[:].bitcast(F32R))
            pm = ps_mm.tile([P, E], F32, tag="mm")
            nmm = KT + (1 if has_bias else 0)
            for k in range(KT):
                nc.tensor.matmul(pm[:], x[k][:, t * P:(t + 1) * P], w["wv"][k][:],
                                 start=(k == 0), stop=(k == nmm - 1))
            if has_bias:
                nc.tensor.matmul(pm[:], ones[:, 0:P], bt["bv"][:],
                                 start=False, stop=True)
            dst = vhat[t][:].rearrange("p (h c) -> p h c", c=DH + 1)[:, :, 0:DH]
            src = pm[:].rearrange("p (h c) -> p h c", c=DH)
            nc.vector.tensor_copy(dst, src)

        # ---- projection emission helpers ---------------------------------
        # One "group" = the 4(+bias) accumulating matmuls for a 128x512
        # slice of one projection, plus its PSUM->SBUF evacuating cast.
        # Pair-0 groups are emitted up front; later pairs' groups are used
        # as PE filler inside the head loop so the PE never starves while
        # ACT chews through the exps.
        def proj_group(j, wname, bname, dest, nh):
            cols = slice(nh * 512, (nh + 1) * 512)
            pm = ps_mm.tile([P, 512], F32, tag="mm", name="pm")
            nmm = KT + (1 if has_bias else 0)
            for k in range(KT):
                nc.tensor.matmul(pm[:], w[wname][k][:, j * P:(j + 1) * P],
                                 x[k][:, cols],
                                 start=(k == 0), stop=(k == nmm - 1))
            if has_bias:
                nc.tensor.matmul(pm[:], bt[bname][:, j * P:(j + 1) * P],
                                 ones[:, 0:512], start=False, stop=True)
            nc.vector.tensor_copy(dest[j][:, cols], pm[:])

        PROJS = (("wq", "bq", qT), ("wk", "bk", kTt),
                 ("wqs", "bqs", qsT), ("wks", "bks", ksT))
        filler = []  # queue of pending proj-group closures for pairs 1..3
        for j in range(1, MT):
            for wname, bname, dest in PROJS:
                for nh in range(NHALF):
                    filler.append((j, wname, bname, dest, nh))

        def emit_filler(n):
            for _ in range(n):
                if filler:
                    proj_group(*filler.pop(0))

        def flush_filler_for_pair(j):
            # everything pair j needs must be emitted before its heads run
            keep = [f for f in filler if f[0] > j]
            need = [f for f in filler if f[0] <= j]
            filler[:] = keep
            for f in need:
                proj_group(*f)

        for wname, bname, dest in PROJS:
            for nh in range(NHALF):
                proj_group(0, wname, bname, dest, nh)

        # ---- head loop ----------------------------------------------------
        # Per head: scores (with in-place self-score diagonal overwrite),
        # exp on ACT, attention matmuls delayed by one s-tile so they never
        # wait on ACT. Normalization is lagged by one head to keep its
        # broadcast matmul off the PE critical path.
        def normalize(h):
            j, hh = divmod(h, 2)
            po = hh * DH
            rcb_ps = ps_sc.tile([DH, L], F32, tag="sc", name="rcb_ps")
            for nh in range(NHALF):
                cols = slice(nh * 512, (nh + 1) * 512)
                nc.tensor.matmul(rcb_ps[:, cols], ones64[:], sums[h][:, cols],
                                 start=True, stop=True)
            # recip must write at base partition 0 (custom-DVE ops silently
            # corrupt cross-base outputs); the final mul's two SBUF inputs
            # (au, rcb) are base 0, only the output lands at base po.
            rcb = psm.tile([DH, L], F32, tag="rcb", name="rcb")
            nc.vector.reciprocal_approx_fast(rcb[:], rcb_ps[:])
            nc.vector.tensor_tensor(attnT[j][po:po + DH, :], au[h][:],
                                    rcb[:], op=AluOpType.mult)

        sums = [None] * H
        au = [None] * H
        for j in range(MT):
            flush_filler_for_pair(j)
            for hh in range(2):
                h = 2 * j + hh
                po = hh * DH  # partition offset of this head inside pair tiles
                at = ps_at.tile([P, L], F32, tag="at", name="at")

                def attn_mms(t):
                    ex = exps[t % 4]
                    for nh in range(NHALF):
                        cols = slice(nh * 512, (nh + 1) * 512)
                        nc.tensor.matmul(at[0:DH + 1, cols],
                                         vhat[t][:, h * (DH + 1):(h + 1) * (DH + 1)],
                                         ex[:, cols],
                                         start=(t == 0), stop=(t == ST - 1))

                def self_mms(t):
                    # agent-diagonal blocks: overwrite with self scores.
                    # Delayed one s-tile so the same-bank WAW after the big
                    # score matmuls never stalls the PE.
                    sc = scs[t % 2]
                    for b in range(2):
                        cs = slice(t * P + b * DH, t * P + (b + 1) * DH)
                        nc.tensor.matmul(sc[b * DH:(b + 1) * DH, cs],
                                         ksT[j][po:po + DH, cs],
                                         qsT[j][po:po + DH, cs],
                                         start=True, stop=True,
                                         tile_position=(po, b * DH))
                    if has_mask:
                        mk = pmk.tile([P, L], F32, tag="mk", name="mk")
                        nc.sync.dma_start(mk[:], mask_d[t * P:(t + 1) * P, :])
                        nc.vector.tensor_tensor(sc[:], sc[:], mk[:],
                                                op=AluOpType.add)
                    ex = pexp.tile([P, L], F32R, tag="exp", name="ex")
                    nc.scalar.activation(ex[:], sc[:], EXP)
                    exps[t % 4] = ex

                exps = [None] * 4
                scs = [None] * 2
                for t in range(ST):
                    sc = ps_sc.tile([P, L], F32, tag="sc", name="sc")
                    scs[t % 2] = sc
                    for nh in range(NHALF):
                        cols = slice(nh * 512, (nh + 1) * 512)
                        nc.tensor.matmul(sc[:, cols],
                                         kTt[j][po:po + DH, t * P:(t + 1) * P],
                                         qT[j][po:po + DH, cols],
                                         start=True, stop=True,
                                         tile_position=(po, 0))
                    if t >= 1:
                        self_mms(t - 1)
                    if t in (1, 5):
                        emit_filler(1)
                    if t >= 2:
                        attn_mms(t - 2)
                    if t == 3:
                        if h >= 1:
                            normalize(h - 1)
                        else:
                            emit_filler(1)
                self_mms(ST - 1)
                attn_mms(ST - 2)
                attn_mms(ST - 1)
                # evacuate attention PSUM: rows 0:64 -> au staging (base 0),
                # row 64 -> sums
                au[h] = psm.tile([DH, L], F32, tag="au", bufs=2,
                                 name=f"au{h}")
                nc.vector.tensor_copy(au[h][:], at[0:DH, :])
                sums[h] = psm.tile([1, L], FP16, tag="sums", bufs=3,
                                   name=f"sums{h}")
                nc.vector.tensor_copy(sums[h][:], at[DH:DH + 1, :])
        normalize(H - 1)

        # ---- output projection -------------------------------------------
        for m in range(MT):
            for nh in range(NHALF):
                cols = slice(nh * 512, (nh + 1) * 512)
                pm = ps_mm.tile([P, 512], F32, tag="mm", name="pm_o")
                for k in range(KT):
                    nc.tensor.matmul(pm[:], w["wout"][k][:, m * P:(m + 1) * P],
                                     attnT[k][:, cols],
                                     start=(k == 0), stop=(k == KT - 1))
                nc.vector.tensor_copy(outT[m][:, cols], pm[:])
            nc.sync.dma_start(out_d[m * P:(m + 1) * P, :], outT[m][:])

    nc.compile()
    return nc


def _get_program(has_bias, has_mask):
    key = (has_bias, has_mask)
    if key not in _PROG_CACHE:
        _PROG_CACHE[key] = _build_program(has_bias, has_mask)
    return _PROG_CACHE[key]


def kernel(**inputs):
    query = np.asarray(inputs["query"], np.float32)
    W = np.asarray(inputs["in_proj_weight"], np.float32)
    b = np.asarray(inputs["in_proj_bias"], np.float32)
    Ws = np.asarray(inputs["in_proj_weight_self"], np.float32)
    bs = np.asarray(inputs["in_proj_bias_self"], np.float32)
    Wo = np.asarray(inputs["out_proj_weight"], np.float32)
    bo = np.asarray(inputs["out_proj_bias"], np.float32)
    mask = np.asarray(inputs["attn_mask"], np.float32)
    num_agent = int(inputs["num_agent"])
    num_heads = int(inputs["num_heads"])
    assert query.shape == (L, N, E) and num_agent == A and num_heads == H
    scale = np.float32(DH ** -0.5)

    # permute rows by agent: new row a*GPA + g  <-  old row g*A + a
    qp = query.reshape(GPA, A, N, E).transpose(1, 0, 2, 3).reshape(L, N, E)

    Wq, Wk, Wv = W[0:E], W[E:2 * E], W[2 * E:3 * E]
    Wqs, Wks = Ws[0:E], Ws[E:2 * E]
    wmats = {
        "wq": np.ascontiguousarray((Wq * scale).T.astype(np.float16)),
        "wk": np.ascontiguousarray(Wk.T.astype(np.float16)),
        "wv": np.ascontiguousarray(Wv.T.astype(np.float16)),
        "wqs": np.ascontiguousarray((Wqs * scale).T.astype(np.float16)),
        "wks": np.ascontiguousarray(Wks.T.astype(np.float16)),
        "wout": np.ascontiguousarray(Wo.T.astype(np.float16)),
    }
    has_bias = bool(np.any(b) or np.any(bs))
    has_mask = bool(np.any(mask))

    common = dict(wmats)
    common["ones8"] = np.ones((P, H), np.float32)
    common["ones64"] = np.ones((1, DH), np.float16)
    if has_bias:
        bq, bk, bv = b[0:E], b[E:2 * E], b[2 * E:3 * E]
        bqs, bks = bs[0:E], bs[E:2 * E]
        common["bq"] = np.ascontiguousarray((bq * scale).reshape(1, E).astype(np.float16))
        common["bk"] = np.ascontiguousarray(bk.reshape(1, E).astype(np.float16))
        common["bv"] = np.ascontiguousarray(bv.reshape(1, E).astype(np.float16))
        common["bqs"] = np.ascontiguousarray((bqs * scale).reshape(1, E).astype(np.float16))
        common["bks"] = np.ascontiguousarray(bks.reshape(1, E).astype(np.float16))
        common["ones"] = np.ones((1, E), np.float16)
    if has_mask:
        perm = np.arange(L).reshape(GPA, A).T.reshape(L)
        mask_perm = mask[np.ix_(perm, perm)]
        common["mask_t"] = np.ascontiguousarray(mask_perm.T)

    in_maps = []
    for n in range(N):
        m = dict(common)
        m["x_t"] = np.ascontiguousarray(qp[:, n, :].T.astype(np.float16))
        in_maps.append(m)

    nc = _get_program(has_bias, has_mask)
    res = bass_utils.run_bass_kernel_spmd(nc, in_maps, core_ids=list(range(N)))

    out = np.empty((L, N, E), np.float32)
    for n in range(N):
        out[:, n, :] = res.results[n]["out_t"].T
    # inverse agent permutation
    out = out.reshape(A, GPA, N, E).transpose(1, 0, 2, 3).reshape(L, N, E)
    out = out + bo
    return out.astype(np.float32)
